# revision 31
# baseline (speedup 1.0000x reference)
"""Trainium2 Bass kernel for nn_CrossLayer (DCN-style cross stack).

Reference semantics (B=16384, D=1024, L=8):
    out_0 = x
    s_i = einsum('bd,d->b', out_i, W[i])
    out_{i+1} = x * s_i[:, None] + b[i] + x

Algebraic collapse: out_{i+1} = x * rho_{i+1} + b[i] with
    rho_1 = u_0 + 1,   rho_{l+1} = rho_l * u_l + c_l
    u_l[r] = <x[r, :], W[l]>          (U = x @ W.T, [B, L])
    c_l = <b[l-1], W[l]> + 1          (weights-only scalars)
    out = x * rho_8[:, None] + b[L-1]

Device work: U = x @ W.T via PE transposes + W-stationary matmuls, all in
float32r (1-pass PE datapath), an 8-step per-row scan on DVE (initial=1,
c_0=1 folds the +1), one fused scale+bias pass per 128-row slot.

Memory layout: 512-row blocks where partition p holds DRAM rows
4p..4p+3 of the block -> every x/y DMA descriptor is 16KB contiguous,
which maximizes per-DMA-engine throughput (the fabric is
descriptor-rate limited around ~25GB/s/engine at 8KB). The row
permutation is never undone: transposes, scan, fuse and the output DMA
all share the same (p, slot) mapping, slot = row % 4.

Streaming: input x owns the sync/HWDGE queue (all 4 block DMAs issued
up front, bufs=4); constants load via the gpsimd queue; outputs
alternate between the gpsimd and sync queues, and the last block goes
out as four per-slot quarters alternating across both queues to
shorten the drain. x read once, out written once -> memory-roofline
bound.

Sharding: data-parallel over batch; 8 cores x 2048 rows. Tiny (L, D)
weights replicated.
"""

import numpy as np

import concourse.bacc as bacc
import concourse.tile as tile
from concourse import mybir
from concourse.bass_utils import run_bass_kernel_spmd
from concourse.masks import make_identity

N_CORES = 8
B, D, L = 16384, 1024, 8
RPC = B // N_CORES          # rows per core (2048)
R = 4                       # adjacent DRAM rows per partition (16KB descr.)
BR = 128 * R                # rows per block (512)
NB = RPC // BR              # blocks per core (4)
NCH = D // 128              # 128-wide d chunks (8)

LAST_RESULTS = None


def _build(cvals):
    """Trace + compile the per-core program. cvals = [c_1..c_{L-1}]."""
    nc = bacc.Bacc("TRN2", target_bir_lowering=False, debug=False)
    f32 = mybir.dt.float32
    f32r = mybir.dt.float32r

    # x/wt declared f32r (byte-identical to the f32 numpy payload) so they
    # can be DMAd straight into f32r tiles (no cast) and the BIR
    # fp32r-producer check is satisfied.
    x_d = nc.dram_tensor("x", [RPC, D], f32r, kind="ExternalInput")
    wt_d = nc.dram_tensor("wt", [128, NCH * L], f32r, kind="ExternalInput")
    b7_d = nc.dram_tensor("b7r", [128, D], f32, kind="ExternalInput")
    y_d = nc.dram_tensor("y", [RPC, D], f32, kind="ExternalOutput")

    # block views: partition p <-> rows 4p..4p+3 of the block
    x_blk = x_d.ap().rearrange("(t p r) d -> t p (r d)", p=128, r=R)
    y_blk = y_d.ap().rearrange("(t p r) d -> t p (r d)", p=128, r=R)

    with tile.TileContext(nc) as tc:
        with (
            tc.tile_pool(name="const", bufs=1) as cpool,
            tc.tile_pool(name="xp", bufs=4) as xpool,
            tc.tile_pool(name="xtp", bufs=2) as xtpool,
            tc.tile_pool(name="yp", bufs=2) as ypool,
            tc.tile_pool(name="small", bufs=6) as spool,
            tc.tile_pool(name="pst", bufs=2, space="PSUM") as pst,
            tc.tile_pool(name="psu", bufs=2, space="PSUM") as psu,
            tc.tile_pool(name="psr", bufs=2, space="PSUM") as psr,
        ):
            # --- all x input DMAs issued up front on the (otherwise empty)
            # sync queue; bufs=NB so it never stalls on buffer recycling ---
            xbs = []
            for i in range(NB):
                xb = xpool.tile([128, R * D], f32r, tag="xb")
                nc.sync.dma_start(out=xb[:], in_=x_blk[i])
                xbs.append(xb)

            # --- constants via the gpsimd queue (idle until outputs) ---
            b7_sb = cpool.tile([128, D], f32)
            nc.gpsimd.dma_start(out=b7_sb[:], in_=b7_d[:, :])
            wt_sb = cpool.tile([128, NCH, L], f32r)
            nc.gpsimd.dma_start(out=wt_sb[:], in_=wt_d.ap().rearrange("p (c l) -> p c l", l=L))
            # identity built on-chip (fp32), rounded to f32r
            idf = cpool.tile([128, 128], f32)
            make_identity(nc, idf[:])
            ident = cpool.tile([128, 128], f32r)
            nc.scalar.copy(ident[:], idf[:])
            # scan constants: cc[:, 0] = 1 (folds the +1 of rho_1), cc[:, l] = c_l
            cc_sb = cpool.tile([128, L], f32)
            nc.gpsimd.memset(cc_sb[:, 0:1], 1.0)
            for l in range(1, L):
                nc.gpsimd.memset(cc_sb[:, l : l + 1], cvals[l - 1])
            ones = cpool.tile([128, 1], f32)
            nc.gpsimd.memset(ones[:], 1.0)

            for i in range(NB):
                xb = xbs[i]
                # [p, slot, chunk, 128] and [p, slot, 1024] views
                xb_c = xb[:].rearrange("p (r c d) -> p r c d", r=R, c=NCH)
                xb_f = xb[:].rearrange("p (r d) -> p r d", r=R)

                # transpose chunks -> xT [128d, c, R*128]; col = s*128 + p
                xT = xtpool.tile([128, NCH, BR], f32r, tag="xT")
                for s in range(R):
                    off = 128 * s
                    pt = pst.tile([128, NCH, 128], f32, tag="pst")
                    for c in range(NCH):
                        nc.tensor.transpose(
                            pt[:, c, :].bitcast(f32r), xb_c[:, s, c, :], ident[:]
                        )
                    nc.scalar.copy(xT[:, :, off : off + 128], pt[:].bitcast(f32r))

                # U^T for the block: [L, BR] = sum_c WT_c.T @ xT_c
                ps_u = psu.tile([L, BR], f32, tag="psu")
                for c in range(NCH):
                    nc.tensor.matmul(
                        ps_u[:], wt_sb[:, c, :], xT[:, c, :],
                        start=(c == 0), stop=(c == NCH - 1),
                    )
                ut = spool.tile([L, BR], f32r, tag="ut")
                nc.scalar.copy(ut[:], ps_u[:])

                yt = ypool.tile([128, R, D], f32, tag="yt")
                last = i == NB - 1
                for s in range(R):
                    off = 128 * s
                    # U slot back to row-partition orientation: [128, L]
                    pr = psr.tile([128, L], f32, tag="psr")
                    nc.tensor.transpose(
                        pr[:].bitcast(f32r), ut[:, off : off + 128], ident[0:L, 0:L]
                    )
                    # rho chain: rho_{l+1} = rho_l*u_l + c_l, rho_0 = c_0 = 1
                    scano = spool.tile([128, L], f32, tag="scan")
                    nc.vector.tensor_tensor_scan(
                        scano[:], pr[:], cc_sb[:], ones[:, 0:1],
                        mybir.AluOpType.mult, mybir.AluOpType.add,
                    )
                    # out = x * rho + b7
                    nc.vector.scalar_tensor_tensor(
                        yt[:, s, :], xb_f[:, s, :].bitcast(f32),
                        scano[:, L - 1 : L], b7_sb[:],
                        mybir.AluOpType.mult, mybir.AluOpType.add,
                    )
                    if last:
                        # drain the final block as per-slot quarters across
                        # both queues to shorten the tail
                        eng = nc.sync if s % 2 == 0 else nc.gpsimd
                        eng.dma_start(
                            out=y_blk[i][:, D * s : D * (s + 1)], in_=yt[:, s, :]
                        )
                if not last:
                    # alternate output queues: SWDGE (gpsimd) / HWDGE (sync)
                    eng = nc.gpsimd if i % 2 == 0 else nc.sync
                    eng.dma_start(out=y_blk[i], in_=yt[:])

    nc.compile()
    return nc


def kernel(x, W, b):
    global LAST_RESULTS
    x = np.ascontiguousarray(np.asarray(x), dtype=np.float32)
    W = np.ascontiguousarray(np.asarray(W), dtype=np.float32)
    b = np.ascontiguousarray(np.asarray(b), dtype=np.float32)
    assert x.shape == (B, D) and W.shape == (L, D) and b.shape == (L, D)

    cvals = [float(np.dot(b[l - 1].astype(np.float64), W[l].astype(np.float64)) + 1.0)
             for l in range(1, L)]
    wt = W.T.reshape(NCH, 128, L).transpose(1, 0, 2).reshape(128, NCH * L)
    wt = np.ascontiguousarray(wt, dtype=np.float32)
    b7r = np.ascontiguousarray(np.broadcast_to(b[L - 1], (128, D)), dtype=np.float32)

    nc = _build(cvals)

    shards = [x[i * RPC : (i + 1) * RPC] for i in range(N_CORES)]
    in_maps = [{"x": s, "wt": wt, "b7r": b7r} for s in shards]
    res = run_bass_kernel_spmd(nc, in_maps, core_ids=list(range(N_CORES)))
    LAST_RESULTS = res
    out = np.concatenate([res.results[i]["y"] for i in range(N_CORES)], axis=0)
    return out.astype(np.float32)


# revision 32
# speedup vs baseline: 1.0543x; 1.0543x over previous
"""Trainium2 Bass kernel for nn_CrossLayer (DCN-style cross stack).

Reference semantics (B=16384, D=1024, L=8):
    out_0 = x
    s_i = einsum('bd,d->b', out_i, W[i])
    out_{i+1} = x * s_i[:, None] + b[i] + x

Algebraic collapse: out_{i+1} = x * rho_{i+1} + b[i] with
    rho_1 = u_0 + 1,   rho_{l+1} = rho_l * u_l + c_l
    u_l[r] = <x[r, :], W[l]>          (U = x @ W.T, [B, L])
    c_l = <b[l-1], W[l]> + 1          (weights-only scalars)
    out = x * rho_8[:, None] + b[L-1]

Device work: U = x @ W.T via PE transposes + W-stationary matmuls, all in
float32r (1-pass PE datapath: transpose 1.5 cyc/row, matmul 1 cyc/row at
>=256 moving cols, vs 2/4 for plain fp32), an 8-step per-row scan on DVE
(initial=1, c_0=1 folds the +1 into the scan), one fused scale+bias pass
per 128-row slot.

Memory layout: 256-row blocks where partition p holds DRAM rows 2p/2p+1
of the block -> every x/y DMA descriptor is 8KB contiguous (the sweet
spot: the DMA fabric is ~25GB/s per engine there, ~420GB/s aggregate).
The row permutation is never undone: transposes, scan, fuse, and the
output DMA all use the same (p, slot) mapping.

Streaming: input x owns the sync/HWDGE queue exclusively (all 8 block
DMAs issued up front, bufs=8, so the queue never stalls); constants load
via the gpsimd queue; outputs alternate between the gpsimd and sync
queues so the drain is not serialized behind one stream, and the last
block's output goes out as two per-slot halves on both queues. x read
once, out written once -> memory-roofline bound.

Sharding: data-parallel over batch; 8 cores x 2048 rows. Tiny (L, D)
weights replicated.
"""

import numpy as np

import concourse.bacc as bacc
import concourse.tile as tile
from concourse import mybir
from concourse.bass_utils import run_bass_kernel_spmd
from concourse.masks import make_identity

N_CORES = 8
B, D, L = 16384, 1024, 8
RPC = B // N_CORES          # rows per core (2048)
NB = RPC // 256             # 256-row blocks per core (8)
NCH = D // 128              # 128-wide d chunks (8)

LAST_RESULTS = None


def _build(cvals):
    """Trace + compile the per-core program. cvals = [c_1..c_{L-1}]."""
    nc = bacc.Bacc("TRN2", target_bir_lowering=False, debug=False)
    f32 = mybir.dt.float32
    f32r = mybir.dt.float32r

    # x/wt declared f32r (byte-identical to the f32 numpy payload) so they
    # can be DMAd straight into f32r tiles (no cast) and the BIR
    # fp32r-producer check is satisfied.
    x_d = nc.dram_tensor("x", [RPC, D], f32r, kind="ExternalInput")
    wt_d = nc.dram_tensor("wt", [128, NCH * L], f32r, kind="ExternalInput")
    b7_d = nc.dram_tensor("b7r", [128, D], f32, kind="ExternalInput")
    y_d = nc.dram_tensor("y", [RPC, D], f32, kind="ExternalOutput")

    # block views: partition p <-> rows 2p, 2p+1 of the block (8KB descr.)
    x_blk = x_d.ap().rearrange("(t p r) d -> t p (r d)", p=128, r=2)
    y_blk = y_d.ap().rearrange("(t p r) d -> t p (r d)", p=128, r=2)

    with tile.TileContext(nc) as tc:
        with (
            tc.tile_pool(name="const", bufs=1) as cpool,
            tc.tile_pool(name="xp", bufs=8) as xpool,
            tc.tile_pool(name="xtp", bufs=3) as xtpool,
            tc.tile_pool(name="yp", bufs=4) as ypool,
            tc.tile_pool(name="small", bufs=6) as spool,
            tc.tile_pool(name="pst", bufs=2, space="PSUM") as pst,
            tc.tile_pool(name="psu", bufs=2, space="PSUM") as psu,
            tc.tile_pool(name="psr", bufs=2, space="PSUM") as psr,
        ):
            # --- all x input DMAs issued up front on the (otherwise empty)
            # sync queue; bufs=8 so it never stalls on buffer recycling ---
            xbs = []
            for i in range(NB):
                xb = xpool.tile([128, 2 * D], f32r, tag="xb")
                nc.sync.dma_start(out=xb[:], in_=x_blk[i])
                xbs.append(xb)

            # --- constants via the gpsimd queue (idle until outputs) ---
            b7_sb = cpool.tile([128, D], f32)
            nc.gpsimd.dma_start(out=b7_sb[:], in_=b7_d[:, :])
            wt_sb = cpool.tile([128, NCH, L], f32r)
            nc.gpsimd.dma_start(out=wt_sb[:], in_=wt_d.ap().rearrange("p (c l) -> p c l", l=L))
            # identity built on-chip (fp32), rounded to f32r
            idf = cpool.tile([128, 128], f32)
            make_identity(nc, idf[:])
            ident = cpool.tile([128, 128], f32r)
            nc.scalar.copy(ident[:], idf[:])
            # scan constants: cc[:, 0] = 1 (folds the +1 of rho_1), cc[:, l] = c_l
            cc_sb = cpool.tile([128, L], f32)
            nc.gpsimd.memset(cc_sb[:, 0:1], 1.0)
            for l in range(1, L):
                nc.gpsimd.memset(cc_sb[:, l : l + 1], cvals[l - 1])
            ones = cpool.tile([128, 1], f32)
            nc.gpsimd.memset(ones[:], 1.0)

            for i in range(NB):
                xb = xbs[i]
                # [p, slot, chunk, 128] and [p, slot, 1024] views
                xb_c = xb[:].rearrange("p (r c d) -> p r c d", r=2, c=NCH)
                xb_f = xb[:].rearrange("p (r d) -> p r d", r=2)

                # transpose chunks -> xT [128d, c, 256]; col = s*128 + p
                xT = xtpool.tile([128, NCH, 256], f32r, tag="xT")
                for s in range(2):
                    off = 128 * s
                    pt = pst.tile([128, NCH, 128], f32, tag="pst")
                    for c in range(NCH):
                        nc.tensor.transpose(
                            pt[:, c, :].bitcast(f32r), xb_c[:, s, c, :], ident[:]
                        )
                    nc.scalar.copy(xT[:, :, off : off + 128], pt[:].bitcast(f32r))

                # U^T for the block: [L, 256] = sum_c WT_c.T @ xT_c
                ps_u = psu.tile([L, 256], f32, tag="psu")
                for c in range(NCH):
                    nc.tensor.matmul(
                        ps_u[:], wt_sb[:, c, :], xT[:, c, :],
                        start=(c == 0), stop=(c == NCH - 1),
                    )
                ut = spool.tile([L, 256], f32r, tag="ut")
                nc.scalar.copy(ut[:], ps_u[:])

                yt = ypool.tile([128, 2, D], f32, tag="yt")
                last = i == NB - 1
                for s in range(2):
                    off = 128 * s
                    # U slot back to row-partition orientation: [128, L]
                    pr = psr.tile([128, L], f32, tag="psr")
                    nc.tensor.transpose(
                        pr[:].bitcast(f32r), ut[:, off : off + 128], ident[0:L, 0:L]
                    )
                    # rho chain: rho_{l+1} = rho_l*u_l + c_l, rho_0 = c_0 = 1
                    scano = spool.tile([128, L], f32, tag="scan")
                    nc.vector.tensor_tensor_scan(
                        scano[:], pr[:], cc_sb[:], ones[:, 0:1],
                        mybir.AluOpType.mult, mybir.AluOpType.add,
                    )
                    # out = x * rho + b7
                    nc.vector.scalar_tensor_tensor(
                        yt[:, s, :], xb_f[:, s, :].bitcast(f32),
                        scano[:, L - 1 : L], b7_sb[:],
                        mybir.AluOpType.mult, mybir.AluOpType.add,
                    )
                    if last:
                        # drain the final block as two per-slot halves on
                        # separate queues to shorten the tail
                        eng = nc.sync if s == 0 else nc.gpsimd
                        eng.dma_start(
                            out=y_blk[i][:, D * s : D * (s + 1)], in_=yt[:, s, :]
                        )
                if not last:
                    # alternate output queues: SWDGE (gpsimd) / HWDGE (sync)
                    eng = nc.gpsimd if i % 2 == 0 else nc.sync
                    eng.dma_start(out=y_blk[i], in_=yt[:])

    nc.compile()
    return nc


def kernel(x, W, b):
    global LAST_RESULTS
    x = np.ascontiguousarray(np.asarray(x), dtype=np.float32)
    W = np.ascontiguousarray(np.asarray(W), dtype=np.float32)
    b = np.ascontiguousarray(np.asarray(b), dtype=np.float32)
    assert x.shape == (B, D) and W.shape == (L, D) and b.shape == (L, D)

    cvals = [float(np.dot(b[l - 1].astype(np.float64), W[l].astype(np.float64)) + 1.0)
             for l in range(1, L)]
    wt = W.T.reshape(NCH, 128, L).transpose(1, 0, 2).reshape(128, NCH * L)
    wt = np.ascontiguousarray(wt, dtype=np.float32)
    b7r = np.ascontiguousarray(np.broadcast_to(b[L - 1], (128, D)), dtype=np.float32)

    nc = _build(cvals)

    shards = [x[i * RPC : (i + 1) * RPC] for i in range(N_CORES)]
    in_maps = [{"x": s, "wt": wt, "b7r": b7r} for s in shards]
    res = run_bass_kernel_spmd(nc, in_maps, core_ids=list(range(N_CORES)))
    LAST_RESULTS = res
    out = np.concatenate([res.results[i]["y"] for i in range(N_CORES)], axis=0)
    return out.astype(np.float32)
